# revision 16
# baseline (speedup 1.0000x reference)
"""Trainium2 Bass kernel for a 4-layer LIF spiking net scanned over T=32 steps.

Strategy (data-parallel, 8 cores):
  - Shard batch B=2048 -> 256 per core; weights replicated.
  - Feature-on-partitions [h, b] layout: every matmul's stationary operand
    is a static weight tile, spikes are the moving operand, zero on-device
    transposes. For H=256 layers the two 128-row h-tiles are FOLDED into
    the free dim: state tiles are [128, 2*BC] with h = tau*128 + p, so the
    LIF elementwise ops run once per layer per step, and the folded s
    tile's column blocks are exactly the next layer's matmul k-tile
    moving operands.
  - Per layer/step: PE does only the W matmuls (fp32, PSUM). ScalarE
    copies PSUM->SBUF fusing the per-partition bias. VectorE does the
    membrane update in two fused scalar_tensor_tensor ops
    (u = beta*m_prev + c; m = (-thr)*s_prev + u) and s = (m > thr).
    reset_t == s_{t-1}, so no extra heaviside. m(0) = c(0) exactly.
  - Layer-1 matmuls depend only on x and run 2 timesteps per call (N=512).
  - c/m/s stage G=4 timesteps in SBUF, then leave as single fully
    contiguous ~1MB DMAs into [T/G, 128, G, 2, BC] scratch; spikes are
    cast f32->uint8 in-flight by SWDGE. Host unpacks to [T, B, H].
"""

import sys

if "/opt/trn_rl_repo" not in sys.path:
    sys.path.insert(0, "/opt/trn_rl_repo")

import numpy as np

T, B, D, H, D4 = 32, 2048, 48, 256, 10
NCORES = 8
BC = B // NCORES  # 256 batch rows per core
P = 128
G = 4  # timesteps per output staging group
NG = T // G
W3F = [2 * BC, 2 * BC, 2 * BC, BC]  # folded per-step width per layer


def _build(betas, thrs):
    """Build the SPMD Bass program (identical on all cores)."""
    import concourse.mybir as mybir
    from concourse import bacc
    from concourse.tile import TileContext

    f32 = mybir.dt.float32
    u8 = mybir.dt.uint8
    Alu = mybir.AluOpType

    # Bacc (not raw Bass): its compile() runs move_matmul_waits_to_ldweights /
    # generate_event_semaphores, which walrus requires (1 sync-wait per inst).
    nc = bacc.Bacc(target_bir_lowering=False)

    # ---- DRAM I/O ----
    xT_d = nc.dram_tensor("xT", [D, T * BC], f32, kind="ExternalInput")
    w1_d = nc.dram_tensor("w1t", [D, H], f32, kind="ExternalInput")
    w2_d = nc.dram_tensor("w2t", [H, H], f32, kind="ExternalInput")
    w3_d = nc.dram_tensor("w3t", [H, H], f32, kind="ExternalInput")
    w4_d = nc.dram_tensor("w4t", [H, D4], f32, kind="ExternalInput")
    b_d = [
        nc.dram_tensor("b1", [H, 1], f32, kind="ExternalInput"),
        nc.dram_tensor("b2", [H, 1], f32, kind="ExternalInput"),
        nc.dram_tensor("b3", [H, 1], f32, kind="ExternalInput"),
        nc.dram_tensor("b4", [D4, 1], f32, kind="ExternalInput"),
    ]
    # per-core outputs: layers 1-3 [NG, 128, G*2*BC] (h = tau*128 + p),
    # layer 4 [NG, 10, G*BC]; spikes as uint8
    so_d, mo_d, co_d = [], [], []
    for l in range(4):
        pl, wl = (P, G * 2 * BC) if l < 3 else (D4, G * BC)
        so_d.append(nc.dram_tensor(f"s{l + 1}o", [NG, pl, wl], u8, kind="ExternalOutput"))
        mo_d.append(nc.dram_tensor(f"m{l + 1}o", [NG, pl, wl], f32, kind="ExternalOutput"))
        co_d.append(nc.dram_tensor(f"c{l + 1}o", [NG, pl, wl], f32, kind="ExternalOutput"))

    ntiles = [2, 2, 2, 1]
    psz = [P, P, P, D4]

    with TileContext(nc) as tc:
        with (
            tc.tile_pool(name="const", bufs=1) as cpool,
            tc.tile_pool(name="xin", bufs=2) as xpool,
            tc.tile_pool(name="stage", bufs=2) as spool,
            tc.tile_pool(name="psum", bufs=6, space="PSUM") as ppool,
            tc.tile_pool(name="psum1", bufs=2, space="PSUM") as ppool1,
        ):
            # ---- load constants ----
            w1_sb = cpool.tile([D, H], f32, name="w1_sb")
            nc.sync.dma_start(w1_sb[:], w1_d[:])
            w2_sb = [cpool.tile([P, H], f32, name=f"w2_sb{j}") for j in range(2)]
            w3_sb = [cpool.tile([P, H], f32, name=f"w3_sb{j}") for j in range(2)]
            w4_sb = [cpool.tile([P, D4], f32, name=f"w4_sb{j}") for j in range(2)]
            for j in range(2):
                nc.sync.dma_start(w2_sb[j][:], w2_d[j * P : (j + 1) * P, :])
                nc.sync.dma_start(w3_sb[j][:], w3_d[j * P : (j + 1) * P, :])
                nc.sync.dma_start(w4_sb[j][:], w4_d[j * P : (j + 1) * P, :])
            wk = [[w1_sb], w2_sb, w3_sb, w4_sb]
            b_sb = []
            for l in range(4):
                cols = []
                for tau in range(ntiles[l]):
                    t_ = cpool.tile([psz[l], 1], f32, name=f"b_sb{l}_{tau}")
                    nc.sync.dma_start(t_[:], b_d[l][tau * psz[l] : (tau + 1) * psz[l], :])
                    cols.append(t_)
                b_sb.append(cols)

            m_prev = [None] * 4  # folded [psz, W3F[l]] slices
            s_prev = [None] * 4

            for tg in range(NG):
                xg = xpool.tile([D, G * BC], f32, tag="xg", name=f"xg{tg}")
                nc.sync.dma_start(xg[:], xT_d[:, tg * G * BC : (tg + 1) * G * BC])

                cst, mst, sst = [], [], []
                for l in range(4):
                    pl, wl = psz[l], G * W3F[l]
                    cst.append(spool.tile([pl, wl], f32, tag=f"c{l}", name=f"c{l}_{tg}"))
                    mst.append(spool.tile([pl, wl], f32, tag=f"m{l}", name=f"m{l}_{tg}"))
                    sst.append(spool.tile([pl, wl], f32, tag=f"s{l}", name=f"s{l}_{tg}"))

                # layer-1 matmuls: x-only dependency, 2 timesteps per call
                for gp in range(0, G, 2):
                    for tau in range(2):
                        sl = slice(tau * P, (tau + 1) * P)
                        ps = ppool1.tile([P, 2 * BC], f32, tag="ps1", name=f"ps1_{tau}")
                        nc.tensor.matmul(
                            ps[:], w1_sb[:, sl], xg[:, gp * BC : (gp + 2) * BC], start=True, stop=True
                        )
                        for dg in range(2):
                            g = gp + dg
                            nc.scalar.add(
                                cst[0][:, g * 512 + tau * BC : g * 512 + (tau + 1) * BC],
                                ps[:, dg * BC : (dg + 1) * BC],
                                b_sb[0][tau][:],
                            )

                for g in range(G):
                    for l in range(4):
                        wl = W3F[l]
                        base = g * wl
                        c = cst[l][:, base : base + wl]
                        m = mst[l][:, base : base + wl]
                        s = sst[l][:, base : base + wl]
                        if l > 0:
                            # rhs: previous layer's folded s for this step;
                            # column block j is k-tile j
                            rb = g * W3F[l - 1]
                            for tau in range(ntiles[l]):
                                sl = slice(tau * psz[l], (tau + 1) * psz[l])
                                ps = ppool.tile([psz[l], BC], f32, tag="ps", name=f"ps{l}_{tau}")
                                for j in range(2):
                                    nc.tensor.matmul(
                                        ps[:],
                                        wk[l][j][:, sl],
                                        sst[l - 1][:, rb + j * BC : rb + (j + 1) * BC],
                                        start=(j == 0),
                                        stop=(j == 1),
                                    )
                                nc.scalar.add(
                                    cst[l][:, base + tau * BC : base + (tau + 1) * BC],
                                    ps[:],
                                    b_sb[l][tau][:],
                                )
                        if m_prev[l] is None:
                            nc.vector.tensor_copy(m, c)  # m(0) = c(0)
                        else:
                            u = spool.tile([psz[l], wl], f32, tag=f"u{l}", bufs=2, name=f"u{l}")
                            nc.vector.scalar_tensor_tensor(
                                u[:], m_prev[l], betas[l], c, Alu.mult, Alu.add
                            )
                            nc.vector.scalar_tensor_tensor(
                                m, s_prev[l], -thrs[l], u[:], Alu.mult, Alu.add
                            )
                        nc.vector.tensor_scalar(s, m, thrs[l], None, Alu.is_gt)
                        m_prev[l] = m
                        s_prev[l] = s

                # group output DMAs: fully contiguous, ~1MB for layers 1-3.
                # m/c across the two HWDGE rings; s via SWDGE with f32->u8 cast.
                for l in range(4):
                    nc.sync.dma_start(co_d[l][tg], cst[l][:])
                    nc.scalar.dma_start(mo_d[l][tg], mst[l][:])
                    nc.gpsimd.dma_start(so_d[l][tg], sst[l][:])

    nc.compile()
    return nc


LAST = None  # last BassKernelResults (for test harness: exec_time_ns, trace)
NC = None


def kernel(**inputs):
    import os

    from concourse.bass_utils import run_bass_kernel_spmd

    x = np.asarray(inputs["x"], np.float32)
    Ws = [np.asarray(inputs[f"W{i}"], np.float32) for i in (1, 2, 3, 4)]
    bs = [np.asarray(inputs[f"b{i}"], np.float32) for i in (1, 2, 3, 4)]
    betas = [float(np.clip(np.float32(inputs[f"beta{i}"]), 0.0, 1.0)) for i in (1, 2, 3, 4)]
    thrs = [float(np.float32(inputs[f"thr{i}"])) for i in (1, 2, 3, 4)]

    nc = _build(betas, thrs)
    global NC
    NC = nc

    shared = {
        "w1t": np.ascontiguousarray(Ws[0].T),
        "w2t": np.ascontiguousarray(Ws[1].T),
        "w3t": np.ascontiguousarray(Ws[2].T),
        "w4t": np.ascontiguousarray(Ws[3].T),
        "b1": np.ascontiguousarray(bs[0].reshape(H, 1)),
        "b2": np.ascontiguousarray(bs[1].reshape(H, 1)),
        "b3": np.ascontiguousarray(bs[2].reshape(H, 1)),
        "b4": np.ascontiguousarray(bs[3].reshape(D4, 1)),
    }
    in_maps = []
    for c in range(NCORES):
        xc = x[c * BC : (c + 1) * BC]  # [BC, T, D]
        xT = np.ascontiguousarray(xc.transpose(2, 1, 0).reshape(D, T * BC))
        m = dict(shared)
        m["xT"] = xT
        in_maps.append(m)

    kwargs = {}
    if os.environ.get("KTRACE"):
        kwargs["trace"] = True
        if os.environ.get("KTRACE_DIR"):
            kwargs["tmpdir"] = os.environ["KTRACE_DIR"]
    res = run_bass_kernel_spmd(nc, in_maps, core_ids=list(range(NCORES)), **kwargs)
    global LAST
    LAST = res
    results = res.results

    outs = []
    for kind in ("s", "m", "c"):
        for l in range(4):
            hl = H if l < 3 else D4
            full = np.empty((T, B, hl), np.float32)
            for c in range(NCORES):
                dev = results[c][f"{kind}{l + 1}o"]
                if dev.dtype != np.float32:
                    dev = dev.astype(np.float32)
                if l < 3:
                    # [NG, 128, G, 2, BC] -> [T, BC, 256] with h = tau*128 + p
                    part = dev.reshape(NG, P, G, 2, BC).transpose(0, 2, 4, 3, 1).reshape(T, BC, hl)
                else:
                    part = dev.reshape(NG, D4, G, BC).transpose(0, 2, 3, 1).reshape(T, BC, hl)
                full[:, c * BC : (c + 1) * BC, :] = part
            outs.append(full)
    # reference order: (s1..s4, m1..m4, c1..c4)
    return tuple(outs)


if __name__ == "__main__":
    pass


# revision 20
# speedup vs baseline: 1.7391x; 1.7391x over previous
"""Trainium2 Bass kernel for a 4-layer LIF spiking net scanned over T=32 steps.

Strategy (data-parallel, 8 cores):
  - Shard batch B=2048 -> 256 per core; weights replicated.
  - Feature-on-partitions [h, b] layout: every matmul's stationary operand
    is a static weight tile, spikes are the moving operand, zero on-device
    transposes. For H=256 layers the two 128-row h-tiles are FOLDED into
    the free dim: state tiles are [128, 2*BC] with h = tau*128 + p, so the
    LIF elementwise ops run once per layer per step, and the folded s
    tile's column blocks are exactly the next layer's matmul k-tile
    moving operands.
  - Per layer/step: PE does only the W matmuls (fp32, PSUM). ScalarE
    copies PSUM->SBUF fusing the per-partition bias. VectorE does the
    membrane update in two fused scalar_tensor_tensor ops
    (u = beta*m_prev + c; m = (-thr)*s_prev + u) and s = (m > thr).
    reset_t == s_{t-1}, so no extra heaviside. m(0) = c(0) exactly.
  - Layer-1 matmuls depend only on x and run 2 timesteps per call (N=512).
  - c/m/s stage G=4 timesteps in SBUF, then leave as single fully
    contiguous ~1MB DMAs into [T/G, 128, G, 2, BC] scratch; spikes are
    cast f32->uint8 in-flight by SWDGE. Host unpacks to [T, B, H].
"""

import sys

if "/opt/trn_rl_repo" not in sys.path:
    sys.path.insert(0, "/opt/trn_rl_repo")

import numpy as np

T, B, D, H, D4 = 32, 2048, 48, 256, 10
NCORES = 8
BC = B // NCORES  # 256 batch rows per core
P = 128
G = 4  # timesteps per output staging group
NG = T // G
W3F = [2 * BC, 2 * BC, 2 * BC, BC]  # folded per-step width per layer


def _build(betas, thrs):
    """Build the SPMD Bass program (identical on all cores)."""
    import concourse.mybir as mybir
    from concourse import bacc
    from concourse.tile import TileContext

    f32 = mybir.dt.float32
    u8 = mybir.dt.uint8
    Alu = mybir.AluOpType

    # Bacc (not raw Bass): its compile() runs move_matmul_waits_to_ldweights /
    # generate_event_semaphores, which walrus requires (1 sync-wait per inst).
    nc = bacc.Bacc(target_bir_lowering=False)

    # ---- DRAM I/O ----
    xT_d = nc.dram_tensor("xT", [D, T * BC], f32, kind="ExternalInput")
    w1_d = nc.dram_tensor("w1t", [D, H], f32, kind="ExternalInput")
    w2_d = nc.dram_tensor("w2t", [H, H], f32, kind="ExternalInput")
    w3_d = nc.dram_tensor("w3t", [H, H], f32, kind="ExternalInput")
    w4_d = nc.dram_tensor("w4t", [H, D4], f32, kind="ExternalInput")
    b_d = [
        nc.dram_tensor("b1", [H, 1], f32, kind="ExternalInput"),
        nc.dram_tensor("b2", [H, 1], f32, kind="ExternalInput"),
        nc.dram_tensor("b3", [H, 1], f32, kind="ExternalInput"),
        nc.dram_tensor("b4", [D4, 1], f32, kind="ExternalInput"),
    ]
    # per-core outputs: layers 1-3 [NG, 128, G*2*BC] (h = tau*128 + p),
    # layer 4 [NG, 10, G*BC]; spikes as uint8
    so_d, mo_d, co_d = [], [], []
    for l in range(4):
        pl, wl = (P, G * 2 * BC) if l < 3 else (D4, G * BC)
        so_d.append(nc.dram_tensor(f"s{l + 1}o", [NG, pl, wl], u8, kind="ExternalOutput"))
        mo_d.append(nc.dram_tensor(f"m{l + 1}o", [NG, pl, wl], f32, kind="ExternalOutput"))
        co_d.append(nc.dram_tensor(f"c{l + 1}o", [NG, pl, wl], f32, kind="ExternalOutput"))

    ntiles = [2, 2, 2, 1]
    psz = [P, P, P, D4]

    with TileContext(nc) as tc:
        with (
            tc.tile_pool(name="const", bufs=1) as cpool,
            tc.tile_pool(name="xin", bufs=2) as xpool,
            tc.tile_pool(name="stage", bufs=2) as spool,
            tc.tile_pool(name="psum", bufs=6, space="PSUM") as ppool,
            tc.tile_pool(name="psum1", bufs=2, space="PSUM") as ppool1,
        ):
            # ---- load constants ----
            w1_sb = cpool.tile([D, H], f32, name="w1_sb")
            nc.sync.dma_start(w1_sb[:], w1_d[:])
            w2_sb = [cpool.tile([P, H], f32, name=f"w2_sb{j}") for j in range(2)]
            w3_sb = [cpool.tile([P, H], f32, name=f"w3_sb{j}") for j in range(2)]
            w4_sb = [cpool.tile([P, D4], f32, name=f"w4_sb{j}") for j in range(2)]
            for j in range(2):
                nc.sync.dma_start(w2_sb[j][:], w2_d[j * P : (j + 1) * P, :])
                nc.sync.dma_start(w3_sb[j][:], w3_d[j * P : (j + 1) * P, :])
                nc.sync.dma_start(w4_sb[j][:], w4_d[j * P : (j + 1) * P, :])
            wk = [[w1_sb], w2_sb, w3_sb, w4_sb]
            b_sb = []
            for l in range(4):
                cols = []
                for tau in range(ntiles[l]):
                    t_ = cpool.tile([psz[l], 1], f32, name=f"b_sb{l}_{tau}")
                    nc.sync.dma_start(t_[:], b_d[l][tau * psz[l] : (tau + 1) * psz[l], :])
                    cols.append(t_)
                b_sb.append(cols)

            m_prev = [None] * 4  # folded [psz, W3F[l]] slices
            s_prev = [None] * 4
            xgs = [None] * NG
            cst = [[None] * NG for _ in range(4)]
            mst = [[None] * NG for _ in range(4)]
            sst = [[None] * NG for _ in range(4)]

            def unit(l, t):
                """Emit matmuls + LIF for (layer l, step t)."""
                tg, g = divmod(t, G)
                wl = W3F[l]
                if l == 0 and g == 0:
                    # prefetch x one group ahead
                    for tgl in ([tg, tg + 1] if tg == 0 else [tg + 1]):
                        if tgl < NG:
                            xg = xpool.tile([D, G * BC], f32, tag="xg", name=f"xg{tgl}")
                            nc.sync.dma_start(xg[:], xT_d[:, tgl * G * BC : (tgl + 1) * G * BC])
                            xgs[tgl] = xg
                if g == 0:
                    pl, wgl = psz[l], G * wl
                    cst[l][tg] = spool.tile([pl, wgl], f32, tag=f"c{l}", name=f"c{l}_{tg}")
                    mst[l][tg] = spool.tile([pl, wgl], f32, tag=f"m{l}", name=f"m{l}_{tg}")
                    sst[l][tg] = spool.tile([pl, wgl], f32, tag=f"s{l}", name=f"s{l}_{tg}")
                base = g * wl
                if l == 0:
                    if g % 2 == 0:
                        # layer-1 matmuls: x-only dependency, 2 steps per call
                        for tau in range(2):
                            sl = slice(tau * P, (tau + 1) * P)
                            ps = ppool1.tile([P, 2 * BC], f32, tag="ps1", name=f"ps1_{tau}")
                            nc.tensor.matmul(
                                ps[:],
                                w1_sb[:, sl],
                                xgs[tg][:, g * BC : (g + 2) * BC],
                                start=True,
                                stop=True,
                            )
                            for dg in range(2):
                                gg = g + dg
                                nc.scalar.add(
                                    cst[0][tg][:, gg * 512 + tau * BC : gg * 512 + (tau + 1) * BC],
                                    ps[:, dg * BC : (dg + 1) * BC],
                                    b_sb[0][tau][:],
                                )
                else:
                    rb = g * W3F[l - 1]
                    rtile = sst[l - 1][tg]
                    for tau in range(ntiles[l]):
                        sl = slice(tau * psz[l], (tau + 1) * psz[l])
                        ps = ppool.tile([psz[l], BC], f32, tag="ps", name=f"ps{l}_{tau}")
                        for j in range(2):
                            nc.tensor.matmul(
                                ps[:],
                                wk[l][j][:, sl],
                                rtile[:, rb + j * BC : rb + (j + 1) * BC],
                                start=(j == 0),
                                stop=(j == 1),
                            )
                        nc.scalar.add(
                            cst[l][tg][:, base + tau * BC : base + (tau + 1) * BC],
                            ps[:],
                            b_sb[l][tau][:],
                        )
                c = cst[l][tg][:, base : base + wl]
                m = mst[l][tg][:, base : base + wl]
                s = sst[l][tg][:, base : base + wl]
                if m_prev[l] is None:
                    nc.vector.tensor_copy(m, c)  # m(0) = c(0)
                else:
                    u = spool.tile([psz[l], wl], f32, tag=f"u{l}", bufs=3, name=f"u{l}")
                    nc.vector.scalar_tensor_tensor(u[:], m_prev[l], betas[l], c, Alu.mult, Alu.add)
                    nc.vector.scalar_tensor_tensor(m, s_prev[l], -thrs[l], u[:], Alu.mult, Alu.add)
                nc.vector.tensor_scalar(s, m, thrs[l], None, Alu.is_gt)
                m_prev[l] = m
                s_prev[l] = s
                if g == G - 1:
                    # group complete: fully contiguous DMAs (~1MB for l<3).
                    # m/c across the two HWDGE rings; s via SWDGE f32->u8 cast.
                    nc.sync.dma_start(co_d[l][tg], cst[l][tg][:])
                    nc.scalar.dma_start(mo_d[l][tg], mst[l][tg][:])
                    nc.gpsimd.dma_start(so_d[l][tg], sst[l][tg][:])

            # diagonal wavefront: wave w holds up to 4 independent (l, t)
            # units, so every engine always has ready work queued.
            for w in range(T + 3):
                for l in range(4):
                    t = w - l
                    if 0 <= t < T:
                        unit(l, t)

    nc.compile()
    return nc


LAST = None  # last BassKernelResults (for test harness: exec_time_ns, trace)
NC = None


def kernel(**inputs):
    import os

    from concourse.bass_utils import run_bass_kernel_spmd

    x = np.asarray(inputs["x"], np.float32)
    Ws = [np.asarray(inputs[f"W{i}"], np.float32) for i in (1, 2, 3, 4)]
    bs = [np.asarray(inputs[f"b{i}"], np.float32) for i in (1, 2, 3, 4)]
    betas = [float(np.clip(np.float32(inputs[f"beta{i}"]), 0.0, 1.0)) for i in (1, 2, 3, 4)]
    thrs = [float(np.float32(inputs[f"thr{i}"])) for i in (1, 2, 3, 4)]

    nc = _build(betas, thrs)
    global NC
    NC = nc

    shared = {
        "w1t": np.ascontiguousarray(Ws[0].T),
        "w2t": np.ascontiguousarray(Ws[1].T),
        "w3t": np.ascontiguousarray(Ws[2].T),
        "w4t": np.ascontiguousarray(Ws[3].T),
        "b1": np.ascontiguousarray(bs[0].reshape(H, 1)),
        "b2": np.ascontiguousarray(bs[1].reshape(H, 1)),
        "b3": np.ascontiguousarray(bs[2].reshape(H, 1)),
        "b4": np.ascontiguousarray(bs[3].reshape(D4, 1)),
    }
    in_maps = []
    for c in range(NCORES):
        xc = x[c * BC : (c + 1) * BC]  # [BC, T, D]
        xT = np.ascontiguousarray(xc.transpose(2, 1, 0).reshape(D, T * BC))
        m = dict(shared)
        m["xT"] = xT
        in_maps.append(m)

    kwargs = {}
    if os.environ.get("KTRACE"):
        kwargs["trace"] = True
        if os.environ.get("KTRACE_DIR"):
            kwargs["tmpdir"] = os.environ["KTRACE_DIR"]
    res = run_bass_kernel_spmd(nc, in_maps, core_ids=list(range(NCORES)), **kwargs)
    global LAST
    LAST = res
    results = res.results

    outs = []
    for kind in ("s", "m", "c"):
        for l in range(4):
            hl = H if l < 3 else D4
            full = np.empty((T, B, hl), np.float32)
            for c in range(NCORES):
                dev = results[c][f"{kind}{l + 1}o"]
                if dev.dtype != np.float32:
                    dev = dev.astype(np.float32)
                if l < 3:
                    # [NG, 128, G, 2, BC] -> [T, BC, 256] with h = tau*128 + p
                    part = dev.reshape(NG, P, G, 2, BC).transpose(0, 2, 4, 3, 1).reshape(T, BC, hl)
                else:
                    part = dev.reshape(NG, D4, G, BC).transpose(0, 2, 3, 1).reshape(T, BC, hl)
                full[:, c * BC : (c + 1) * BC, :] = part
            outs.append(full)
    # reference order: (s1..s4, m1..m4, c1..c4)
    return tuple(outs)


if __name__ == "__main__":
    pass


# revision 23
# speedup vs baseline: 1.8369x; 1.0562x over previous
"""Trainium2 Bass kernel for a 4-layer LIF spiking net scanned over T=32 steps.

Strategy (data-parallel, 8 cores):
  - Shard batch B=2048 -> 256 per core; weights replicated.
  - Feature-on-partitions [h, b] layout: every matmul's stationary operand
    is a static weight tile, spikes are the moving operand, zero on-device
    transposes. For H=256 layers the two 128-row h-tiles are FOLDED into
    the free dim: state tiles are [128, 2*BC] with h = tau*128 + p, so the
    LIF elementwise ops run once per layer per step, and the folded s
    tile's column blocks are exactly the next layer's matmul k-tile
    moving operands.
  - Per layer/step: PE does only the W matmuls (fp32, PSUM). ScalarE
    copies PSUM->SBUF fusing the per-partition bias. VectorE does the
    membrane update in two fused scalar_tensor_tensor ops
    (u = beta*m_prev + c; m = (-thr)*s_prev + u) and s = (m > thr).
    reset_t == s_{t-1}, so no extra heaviside. m(0) = c(0) exactly.
  - Layer-1 matmuls depend only on x and run 2 timesteps per call (N=512).
  - c/m/s stage G=4 timesteps in SBUF, then leave as single fully
    contiguous ~1MB DMAs into [T/G, 128, G, 2, BC] scratch; spikes are
    cast f32->uint8 in-flight by SWDGE. Host unpacks to [T, B, H].
"""

import sys

if "/opt/trn_rl_repo" not in sys.path:
    sys.path.insert(0, "/opt/trn_rl_repo")

import numpy as np

T, B, D, H, D4 = 32, 2048, 48, 256, 10
NCORES = 8
BC = B // NCORES  # 256 batch rows per core
P = 128
G = 4  # timesteps per output staging group
NG = T // G
W3F = [2 * BC, 2 * BC, 2 * BC, BC]  # folded per-step width per layer


def _build(betas, thrs):
    """Build the SPMD Bass program (identical on all cores)."""
    import concourse.mybir as mybir
    from concourse import bacc
    from concourse.tile import TileContext

    f32 = mybir.dt.float32
    u8 = mybir.dt.uint8
    Alu = mybir.AluOpType

    # Bacc (not raw Bass): its compile() runs move_matmul_waits_to_ldweights /
    # generate_event_semaphores, which walrus requires (1 sync-wait per inst).
    nc = bacc.Bacc(target_bir_lowering=False)

    # ---- DRAM I/O ----
    xT_d = nc.dram_tensor("xT", [D, T * BC], f32, kind="ExternalInput")
    w1_d = nc.dram_tensor("w1t", [D, H], f32, kind="ExternalInput")
    w2_d = nc.dram_tensor("w2t", [H, H], f32, kind="ExternalInput")
    w3_d = nc.dram_tensor("w3t", [H, H], f32, kind="ExternalInput")
    w4_d = nc.dram_tensor("w4t", [H, D4], f32, kind="ExternalInput")
    b_d = [
        nc.dram_tensor("b1", [H, 1], f32, kind="ExternalInput"),
        nc.dram_tensor("b2", [H, 1], f32, kind="ExternalInput"),
        nc.dram_tensor("b3", [H, 1], f32, kind="ExternalInput"),
        nc.dram_tensor("b4", [D4, 1], f32, kind="ExternalInput"),
    ]
    # per-core outputs: layers 1-3 [NG, 128, G*2*BC] (h = tau*128 + p),
    # layer 4 [NG, 10, G*BC]; spikes as uint8
    so_d, mo_d, co_d = [], [], []
    for l in range(4):
        pl, wl = (P, G * 2 * BC) if l < 3 else (D4, G * BC)
        so_d.append(nc.dram_tensor(f"s{l + 1}o", [NG, pl, wl], u8, kind="ExternalOutput"))
        mo_d.append(nc.dram_tensor(f"m{l + 1}o", [NG, pl, wl], f32, kind="ExternalOutput"))
        co_d.append(nc.dram_tensor(f"c{l + 1}o", [NG, pl, wl], f32, kind="ExternalOutput"))

    ntiles = [2, 2, 2, 1]
    psz = [P, P, P, D4]

    with TileContext(nc) as tc:
        with (
            tc.tile_pool(name="const", bufs=1) as cpool,
            tc.tile_pool(name="xin", bufs=2) as xpool,
            tc.tile_pool(name="stage", bufs=2) as spool,
            tc.tile_pool(name="psum", bufs=6, space="PSUM") as ppool,
            tc.tile_pool(name="psum1", bufs=2, space="PSUM") as ppool1,
        ):
            # ---- load constants ----
            w1_sb = cpool.tile([D, H], f32, name="w1_sb")
            nc.sync.dma_start(w1_sb[:], w1_d[:])
            w2_sb = [cpool.tile([P, H], f32, name=f"w2_sb{j}") for j in range(2)]
            w3_sb = [cpool.tile([P, H], f32, name=f"w3_sb{j}") for j in range(2)]
            w4_sb = [cpool.tile([P, D4], f32, name=f"w4_sb{j}") for j in range(2)]
            for j in range(2):
                nc.scalar.dma_start(w2_sb[j][:], w2_d[j * P : (j + 1) * P, :])
                nc.gpsimd.dma_start(w3_sb[j][:], w3_d[j * P : (j + 1) * P, :])
                nc.scalar.dma_start(w4_sb[j][:], w4_d[j * P : (j + 1) * P, :])
            wk = [[w1_sb], w2_sb, w3_sb, w4_sb]
            b_sb = []
            for l in range(4):
                cols = []
                for tau in range(ntiles[l]):
                    t_ = cpool.tile([psz[l], 1], f32, name=f"b_sb{l}_{tau}")
                    nc.gpsimd.dma_start(t_[:], b_d[l][tau * psz[l] : (tau + 1) * psz[l], :])
                    cols.append(t_)
                b_sb.append(cols)

            m_prev = [None] * 4  # folded [psz, W3F[l]] slices
            s_prev = [None] * 4
            xgs = [None] * NG
            cst = [[None] * NG for _ in range(4)]
            mst = [[None] * NG for _ in range(4)]
            sst = [[None] * NG for _ in range(4)]

            def unit(l, t):
                """Emit matmuls + LIF for (layer l, step t)."""
                tg, g = divmod(t, G)
                wl = W3F[l]
                if l == 0 and g == 0:
                    # prefetch x one group ahead
                    for tgl in ([tg, tg + 1] if tg == 0 else [tg + 1]):
                        if tgl < NG:
                            xg = xpool.tile([D, G * BC], f32, tag="xg", name=f"xg{tgl}")
                            nc.sync.dma_start(xg[:], xT_d[:, tgl * G * BC : (tgl + 1) * G * BC])
                            xgs[tgl] = xg
                if g == 0:
                    pl, wgl = psz[l], G * wl
                    cst[l][tg] = spool.tile([pl, wgl], f32, tag=f"c{l}", name=f"c{l}_{tg}")
                    mst[l][tg] = spool.tile([pl, wgl], f32, tag=f"m{l}", name=f"m{l}_{tg}")
                    sst[l][tg] = spool.tile([pl, wgl], f32, tag=f"s{l}", name=f"s{l}_{tg}")
                base = g * wl
                if l == 0:
                    if g % 2 == 0:
                        # layer-1 matmuls: x-only dependency, 2 steps per call
                        for tau in range(2):
                            sl = slice(tau * P, (tau + 1) * P)
                            ps = ppool1.tile([P, 2 * BC], f32, tag="ps1", name=f"ps1_{tau}")
                            nc.tensor.matmul(
                                ps[:],
                                w1_sb[:, sl],
                                xgs[tg][:, g * BC : (g + 2) * BC],
                                start=True,
                                stop=True,
                            )
                            for dg in range(2):
                                gg = g + dg
                                nc.scalar.add(
                                    cst[0][tg][:, gg * 512 + tau * BC : gg * 512 + (tau + 1) * BC],
                                    ps[:, dg * BC : (dg + 1) * BC],
                                    b_sb[0][tau][:],
                                )
                else:
                    rb = g * W3F[l - 1]
                    rtile = sst[l - 1][tg]
                    for tau in range(ntiles[l]):
                        sl = slice(tau * psz[l], (tau + 1) * psz[l])
                        ps = ppool.tile([psz[l], BC], f32, tag="ps", name=f"ps{l}_{tau}")
                        for j in range(2):
                            nc.tensor.matmul(
                                ps[:],
                                wk[l][j][:, sl],
                                rtile[:, rb + j * BC : rb + (j + 1) * BC],
                                start=(j == 0),
                                stop=(j == 1),
                            )
                        nc.scalar.add(
                            cst[l][tg][:, base + tau * BC : base + (tau + 1) * BC],
                            ps[:],
                            b_sb[l][tau][:],
                        )
                c = cst[l][tg][:, base : base + wl]
                m = mst[l][tg][:, base : base + wl]
                s = sst[l][tg][:, base : base + wl]
                if m_prev[l] is None:
                    nc.vector.tensor_copy(m, c)  # m(0) = c(0)
                else:
                    u = spool.tile([psz[l], wl], f32, tag=f"u{l}", bufs=3, name=f"u{l}")
                    nc.vector.scalar_tensor_tensor(u[:], m_prev[l], betas[l], c, Alu.mult, Alu.add)
                    nc.vector.scalar_tensor_tensor(m, s_prev[l], -thrs[l], u[:], Alu.mult, Alu.add)
                nc.vector.tensor_scalar(s, m, thrs[l], None, Alu.is_gt)
                m_prev[l] = m
                s_prev[l] = s
                if g % 2 == 1:
                    # half-group complete: contiguous ~512KB DMAs (l<3).
                    # m/c across the two HWDGE rings; s via SWDGE f32->u8 cast.
                    hsl = slice((g - 1) * wl, (g + 1) * wl)
                    nc.sync.dma_start(co_d[l][tg, :, hsl], cst[l][tg][:, hsl])
                    nc.scalar.dma_start(mo_d[l][tg, :, hsl], mst[l][tg][:, hsl])
                    nc.gpsimd.dma_start(so_d[l][tg, :, hsl], sst[l][tg][:, hsl])

            # diagonal wavefront: wave w holds up to 4 independent (l, t)
            # units, so every engine always has ready work queued.
            for w in range(T + 3):
                for l in range(4):
                    t = w - l
                    if 0 <= t < T:
                        unit(l, t)

    nc.compile()
    return nc


LAST = None  # last BassKernelResults (for test harness: exec_time_ns, trace)
NC = None


def kernel(**inputs):
    import os

    from concourse.bass_utils import run_bass_kernel_spmd

    x = np.asarray(inputs["x"], np.float32)
    Ws = [np.asarray(inputs[f"W{i}"], np.float32) for i in (1, 2, 3, 4)]
    bs = [np.asarray(inputs[f"b{i}"], np.float32) for i in (1, 2, 3, 4)]
    betas = [float(np.clip(np.float32(inputs[f"beta{i}"]), 0.0, 1.0)) for i in (1, 2, 3, 4)]
    thrs = [float(np.float32(inputs[f"thr{i}"])) for i in (1, 2, 3, 4)]

    nc = _build(betas, thrs)
    global NC
    NC = nc

    shared = {
        "w1t": np.ascontiguousarray(Ws[0].T),
        "w2t": np.ascontiguousarray(Ws[1].T),
        "w3t": np.ascontiguousarray(Ws[2].T),
        "w4t": np.ascontiguousarray(Ws[3].T),
        "b1": np.ascontiguousarray(bs[0].reshape(H, 1)),
        "b2": np.ascontiguousarray(bs[1].reshape(H, 1)),
        "b3": np.ascontiguousarray(bs[2].reshape(H, 1)),
        "b4": np.ascontiguousarray(bs[3].reshape(D4, 1)),
    }
    in_maps = []
    for c in range(NCORES):
        xc = x[c * BC : (c + 1) * BC]  # [BC, T, D]
        xT = np.ascontiguousarray(xc.transpose(2, 1, 0).reshape(D, T * BC))
        m = dict(shared)
        m["xT"] = xT
        in_maps.append(m)

    kwargs = {}
    if os.environ.get("KTRACE"):
        kwargs["trace"] = True
        if os.environ.get("KTRACE_DIR"):
            kwargs["tmpdir"] = os.environ["KTRACE_DIR"]
    res = run_bass_kernel_spmd(nc, in_maps, core_ids=list(range(NCORES)), **kwargs)
    global LAST
    LAST = res
    results = res.results

    outs = []
    for kind in ("s", "m", "c"):
        for l in range(4):
            hl = H if l < 3 else D4
            full = np.empty((T, B, hl), np.float32)
            for c in range(NCORES):
                dev = results[c][f"{kind}{l + 1}o"]
                if dev.dtype != np.float32:
                    dev = dev.astype(np.float32)
                if l < 3:
                    # [NG, 128, G, 2, BC] -> [T, BC, 256] with h = tau*128 + p
                    part = dev.reshape(NG, P, G, 2, BC).transpose(0, 2, 4, 3, 1).reshape(T, BC, hl)
                else:
                    part = dev.reshape(NG, D4, G, BC).transpose(0, 2, 3, 1).reshape(T, BC, hl)
                full[:, c * BC : (c + 1) * BC, :] = part
            outs.append(full)
    # reference order: (s1..s4, m1..m4, c1..c4)
    return tuple(outs)


if __name__ == "__main__":
    pass
